# revision 1
# baseline (speedup 1.0000x reference)
"""Trainium2 Bass kernel for the Mamba U-Net model (nn_Model_20770461843918).

Batch-data-parallel SPMD over 8 NeuronCores (4 batch elements; cores c and
c+4 duplicate work, outputs read from cores 0-3).  Per core the whole
7-block Mamba U-Net runs locally with partitions = inner channel d:
  PE : all matmuls (in/x/dt/out projections, depthwise conv via diagonal
       matmuls, down/up/gate convs) + K=1 ones-matmul broadcast of the
       per-timestep B/C rows across partitions
  ACT: exp(dt*A) per state n, silu, softplus, sigmoid, PSUM->SBUF copies
  DVE: dBu = (dt*u)*B_rep, selective scan via tensor_tensor_scan
       (h_t = dA_t*h_{t-1} + dBu_t, fp32 state), h*C_rep, tree-reduce over n
"""
import numpy as np

B, L0, C = 4, 1024, 128
DI, NST, R, KC = 256, 16, 8, 4
NV = NST + 3          # packed per-partition vec cols: A[16], D, convb, bdt
NCORES = 8
TS = 512              # scan-stage time chunk
MM = 512              # matmul-stage time chunk

_CACHE = {}


def _prep_weights(inp):
    f32 = np.float32
    g = lambda k: np.asarray(inp[k], f32)
    m_Win, m_convw, m_convb = g("m_Win"), g("m_convw"), g("m_convb")
    m_Wx, m_Wdt, m_bdt = g("m_Wx"), g("m_Wdt"), g("m_bdt")
    m_Alog, m_D, m_Wout = g("m_Alog"), g("m_D"), g("m_Wout")
    dc_w, dc_b = g("dc_w"), g("dc_b")
    wg_W, wg_b, db_W, db_b = g("wg_W"), g("wg_b"), g("db_W"), g("db_b")
    up_w, up_b = g("up_w"), g("up_b")

    w = {}
    w["winT"] = np.ascontiguousarray(m_Win.transpose(0, 2, 1))           # [7, C, 512]
    cd = np.zeros((7, 2, KC, 128, 128), f32)
    idx = np.arange(128)
    for i in range(7):
        for gg in range(2):
            for k in range(KC):
                cd[i, gg, k, idx, idx] = m_convw[i, gg * 128:(gg + 1) * 128, k]
    # sbuf layout [128, (g, k, 128)]: partition = k_in, free-block (g,k) = lhsT
    w["convdiag"] = np.ascontiguousarray(cd.transpose(0, 1, 3, 2, 4)).reshape(7, 2, 128, KC * 128)
    wxT_raw = np.ascontiguousarray(m_Wx.transpose(0, 2, 1)).reshape(7, 2, 128, R + 2 * NST)
    wxT = np.zeros((7, 2, 128, 64), f32)
    wxT[..., :R] = wxT_raw[..., :R]          # dt rows -> psum partitions 0..7
    wxT[..., 32:64] = wxT_raw[..., R:]       # B/C rows -> psum partitions 32..63
    w["wxT"] = wxT
    wdtT = np.ascontiguousarray(m_Wdt.transpose(0, 2, 1))                # [7, R, DI]
    w["wdtall"] = wdtT.transpose(1, 0, 2).reshape(R, 7 * DI)             # [8, 7*256]
    A = -np.exp(m_Alog)                                                  # [7, DI, N]
    vec = np.zeros((7, 2, 128, NV), f32)
    for gg in range(2):
        sl = slice(gg * 128, (gg + 1) * 128)
        vec[:, gg, :, :NST] = A[:, sl, :]
        vec[:, gg, :, NST] = m_D[:, sl]
        vec[:, gg, :, NST + 1] = m_convb[:, sl]
        vec[:, gg, :, NST + 2] = m_bdt[:, sl]
    w["vecs"] = vec
    w["woutT"] = np.ascontiguousarray(m_Wout.transpose(0, 2, 1)).reshape(7, 2, 128, C)
    # dc_w[j, co, ci, k] -> [j, ci, (k, co)]
    w["dcwT"] = np.ascontiguousarray(dc_w.transpose(0, 2, 3, 1)).reshape(3, 128, 3 * 128)
    # up_w[j, ci, co, k] -> [j, ci, (k, co)]
    w["upw"] = np.ascontiguousarray(up_w.transpose(0, 1, 3, 2)).reshape(3, 128, 2 * 128)
    w["wgT"] = np.ascontiguousarray(wg_W.transpose(0, 2, 1)).reshape(3, 2, 128, 128)
    w["dbT"] = np.ascontiguousarray(db_W.transpose(0, 2, 1)).reshape(3, 2, 128, 128)
    gv = np.zeros((3, 128, 4), f32)
    gv[:, :, 0], gv[:, :, 1], gv[:, :, 2], gv[:, :, 3] = dc_b, up_b, wg_b, db_b
    w["gvecs"] = gv
    # pack all [128, X] weight panels into one array (order must match _build)
    panels = []
    for i in range(7):
        panels += [w["wxT"][i, 0], w["wxT"][i, 1],
                   w["vecs"][i, 0], w["vecs"][i, 1],
                   w["woutT"][i, 0], w["woutT"][i, 1]]
    for j in range(3):
        panels += [w["dcwT"][j], w["upw"][j],
                   w["wgT"][j, 0], w["wgT"][j, 1],
                   w["dbT"][j, 0], w["dbT"][j, 1], w["gvecs"][j]]
    w2 = {"winT": w["winT"], "convdiag": w["convdiag"], "wdtall": w["wdtall"],
          "wtpack": np.ascontiguousarray(np.concatenate(panels, axis=1))}
    return w2


def _build():
    import concourse.bacc as bacc
    import concourse.tile as tile
    import concourse.mybir as mybir

    F32 = mybir.dt.float32
    Alu = mybir.AluOpType
    Act = mybir.ActivationFunctionType

    nc = bacc.Bacc("TRN2", target_bir_lowering=False, debug=False,
                   num_devices=NCORES)

    xT_d = nc.declare_dram_parameter("xT", [C, L0], F32, isOutput=False)
    out_d = nc.declare_dram_parameter("out", [C, L0], F32, isOutput=True)
    BLKW, GATW = 422, 1156
    TOTW = 7 * BLKW + 3 * GATW
    dram = {}
    for name, shape in [
        ("winT", [7, C, 2 * DI]), ("convdiag", [7, 2, 128, KC * 128]),
        ("wdtall", [R, 7 * DI]), ("wtpack", [128, TOTW]),
    ]:
        dram[name] = nc.declare_dram_parameter(name, shape, F32, isOutput=False)
    BF16 = mybir.dt.bfloat16
    bc_dram2 = [nc.dram_tensor("bc_bounce0", [2 * NST, L0], BF16),
                nc.dram_tensor("bc_bounce1", [2 * NST, L0], BF16)]

    with tile.TileContext(nc) as tc:
        with tc.tile_pool(name="wt", bufs=1) as wt, \
             tc.tile_pool(name="lvl", bufs=1) as lvl, \
             tc.tile_pool(name="blk", bufs=1) as blk, \
             tc.tile_pool(name="cube", bufs=1) as cube, \
             tc.tile_pool(name="cw", bufs=2) as cw, \
             tc.tile_pool(name="ubuf", bufs=1) as ubuf, \
             tc.tile_pool(name="gw", bufs=2) as gw, \
             tc.tile_pool(name="cwc", bufs=2) as cwc, \
             tc.tile_pool(name="bczp", bufs=1) as bczp, \
             tc.tile_pool(name="mmp", bufs=3, space="PSUM") as mmp, \
             tc.tile_pool(name="xdbp", bufs=1, space="PSUM") as xdbp, \
             tc.tile_pool(name="repp", bufs=2, space="PSUM") as repp:

            ones2 = wt.tile([65, 128], BF16, tag="ones2")
            nc.vector.memset(ones2[0:1, :], 1.0)
            nc.vector.memset(ones2[64:65, :], 1.0)

            def load_blk(i):
                winTb = cw.tile([C, 2 * DI], F32, tag="winT", name=f"winTb{i}")
                nc.scalar.dma_start(winTb[:], dram["winT"][i])
                cdw = cwc.tile([128, 2 * KC * 128], F32, tag="convdiag",
                               name=f"cdw{i}")
                nc.scalar.dma_start(cdw[:, :KC * 128], dram["convdiag"][i, 0])
                nc.scalar.dma_start(cdw[:, KC * 128:], dram["convdiag"][i, 1])
                return cdw, winTb

            preload = {0: load_blk(0)}

            wtall = wt.tile([128, TOTW], F32, tag="wtall")
            nc.scalar.dma_start(wtall[:, :BLKW], dram["wtpack"][:, :BLKW])
            nc.scalar.dma_start(wtall[:, BLKW:], dram["wtpack"][:, BLKW:])
            wdtall = wt.tile([R, 7 * DI], F32, tag="wdtall")
            nc.scalar.dma_start(wdtall[:], dram["wdtall"][:])
            wxTt, wdtTt, vecst, woutTt = [], [], [], []
            for i in range(7):
                o = i * BLKW
                wxTt.append(wtall[:, o:o + 128])
                vecst.append(wtall[:, o + 128:o + 128 + 2 * NV])
                woutTt.append(wtall[:, o + 128 + 2 * NV:o + BLKW])
                wdtTt.append(wdtall[:, i * DI:(i + 1) * DI])
            dcwTt, upwt, wgTt, dbTt, gvecst = [], [], [], [], []
            for j in range(3):
                o = 7 * BLKW + j * GATW
                dcwTt.append(wtall[:, o:o + 384])
                upwt.append(wtall[:, o + 384:o + 640])
                wgTt.append(wtall[:, o + 640:o + 896])
                dbTt.append(wtall[:, o + 896:o + 1152])
                gvecst.append(wtall[:, o + 1152:o + 1156])

            # per-block working tiles (reused across blocks)
            xi = [blk.tile([128, L0 + 3], F32, tag=f"xi{g}", name=f"xi{g}")
                  for g in range(2)]
            y_t = [blk.tile([128, L0], F32, tag=f"y{g}", name=f"y{g}")
                   for g in range(2)]
            xdbR = blk.tile([R, L0], F32, tag="xdbR")
            bc16 = blk.tile([2 * NST, L0], BF16, tag="bc16")
            carry = blk.tile([128, 2 * NST], F32, tag="carry")
            dA_t = cube.tile([128, NST * TS], F32, tag="dA")
            dBu_t = cube.tile([128, NST * TS], F32, tag="dBu")

            def mamba(x_ap, i, Lb, out_ap, out_dma=None):
                cdw, winTb = preload.pop(i) if i in preload else load_blk(i)
                u_t = [ubuf.tile([128, L0], F32, tag=f"u{g}", name=f"u{g}_{i}")
                       for g in range(2)]
                dt_t = [ubuf.tile([128, L0], F32, tag=f"dt{g}", name=f"dt{g}_{i}")
                        for g in range(2)]
                vecs = vecst[i]

                def vcol(g, c):
                    return vecs[:, g * NV + c: g * NV + c + 1]
                # ---- stage M ----
                for c0 in range(0, Lb, MM):
                    F = min(MM, Lb - c0)
                    ztmp = cw.tile([128, MM], F32, tag="dtu", name="ztmpM")
                    for p in range(2):
                        ps = mmp.tile([128, MM], F32, tag="mmps")
                        nc.tensor.matmul(ps[:, :F], winTb[:, p * 128:(p + 1) * 128],
                                         x_ap[:, c0:c0 + F], start=True, stop=True)
                        nc.scalar.activation(xi[p][:, 3 + c0:3 + c0 + F], ps[:, :F], Act.Copy)
                    for g in range(2):
                        ps = mmp.tile([128, MM], F32, tag="mmps")
                        for k in range(KC):
                            nc.tensor.matmul(
                                ps[:, :F],
                                cdw[:, (g * KC + k) * 128:(g * KC + k + 1) * 128],
                                xi[g][:, c0 + k:c0 + k + F],
                                start=(k == 0), stop=(k == KC - 1))
                        nc.scalar.activation(u_t[g][:, c0:c0 + F], ps[:, :F], Act.Identity,
                                             bias=vcol(g, NST + 1))
                        nc.scalar.activation(ztmp[:, :F], ps[:, :F], Act.Sigmoid,
                                             bias=vcol(g, NST + 1))
                        nc.vector.tensor_mul(u_t[g][:, c0:c0 + F], u_t[g][:, c0:c0 + F],
                                             ztmp[:, :F])
                    psx = xdbp.tile([64, MM], F32, tag="xdbps")
                    for g in range(2):
                        nc.tensor.matmul(psx[:, :F],
                                         wxTt[i][:, g * 64:(g + 1) * 64],
                                         u_t[g][:, c0:c0 + F], start=(g == 0), stop=(g == 1))
                    nc.scalar.activation(xdbR[:, c0:c0 + F], psx[:R, :F], Act.Copy)
                    nc.scalar.activation(bc16[:, c0:c0 + F], psx[32:, :F], Act.Copy)
                    for g in range(2):
                        ps = mmp.tile([128, MM], F32, tag="mmps")
                        nc.tensor.matmul(ps[:, :F], wdtTt[i][:, g * 128:(g + 1) * 128],
                                         xdbR[:, c0:c0 + F], start=True, stop=True)
                        nc.scalar.activation(ztmp[:, :F], ps[:, :F], Act.Exp,
                                             bias=vcol(g, NST + 2))
                        nc.scalar.activation(dt_t[g][:, c0:c0 + F], ztmp[:, :F], Act.Ln,
                                             bias=1.0)
                    nc.sync.dma_start(bc_dram2[i % 2][:, c0:c0 + F], bc16[:, c0:c0 + F])
                # ---- stage S ----
                nchunks = (Lb + TS - 1) // TS
                for s in range(nchunks):
                    s0 = s * TS
                    F = min(TS, Lb - s0)
                    bc_dram = bc_dram2[i % 2]
                    bcz = bczp.tile([65, NST * TS], BF16, tag="bcz")
                    nc.sync.dma_start(bcz[0:1, :NST * F], bc_dram[0:NST, s0:s0 + F])
                    nc.sync.dma_start(bcz[64:65, :NST * F], bc_dram[NST:, s0:s0 + F])
                    for g in range(2):
                        dtu = cw.tile([128, TS], F32, tag="dtu")
                        nc.vector.tensor_mul(dtu[:, :F], dt_t[g][:, s0:s0 + F],
                                             u_t[g][:, s0:s0 + F])
                        for n in range(NST):
                            nc.scalar.activation(dA_t[:, n * F:(n + 1) * F],
                                                 dt_t[g][:, s0:s0 + F], Act.Exp,
                                                 scale=vcol(g, n))
                        for np2 in range(NST // 2):
                            n0 = 2 * np2
                            rep = repp.tile([128, 2 * TS], F32, tag="rep")
                            nc.tensor.matmul(rep[:, :F], ones2[0:1, :],
                                             bcz[0:1, n0 * F:(n0 + 1) * F],
                                             start=True, stop=True)
                            nc.tensor.matmul(rep[:, F:2 * F], ones2[0:1, :],
                                             bcz[0:1, (n0 + 1) * F:(n0 + 2) * F],
                                             start=True, stop=True)
                            nc.vector.tensor_mul(
                                dBu_t[:, n0 * F:(n0 + 2) * F].rearrange(
                                    "p (a b) -> p a b", a=2),
                                dtu[:, :F].unsqueeze(1).broadcast_to([128, 2, F]),
                                rep[:, :2 * F].rearrange("p (a b) -> p a b", a=2))
                        for n in range(NST):
                            init = 0.0 if s == 0 else carry[:, g * NST + n:g * NST + n + 1]
                            nc.vector.tensor_tensor_scan(
                                dBu_t[:, n * F:(n + 1) * F],
                                dA_t[:, n * F:(n + 1) * F],
                                dBu_t[:, n * F:(n + 1) * F],
                                init, op0=Alu.mult, op1=Alu.add)
                        if s + 1 < nchunks:
                            nc.vector.tensor_copy(carry[:, g * NST:(g + 1) * NST],
                                                  dBu_t[:, F - 1:NST * F:F])
                        for np2 in range(NST // 2):
                            n0 = 2 * np2
                            rep = repp.tile([128, 2 * TS], F32, tag="rep")
                            nc.tensor.matmul(rep[:, :F], ones2[64:65, :],
                                             bcz[64:65, n0 * F:(n0 + 1) * F],
                                             start=True, stop=True)
                            nc.tensor.matmul(rep[:, F:2 * F], ones2[64:65, :],
                                             bcz[64:65, (n0 + 1) * F:(n0 + 2) * F],
                                             start=True, stop=True)
                            nc.vector.tensor_mul(dA_t[:, n0 * F:(n0 + 2) * F],
                                                 dBu_t[:, n0 * F:(n0 + 2) * F],
                                                 rep[:, :2 * F])
                        nc.vector.tensor_add(dA_t[:, :8 * F], dA_t[:, :8 * F], dA_t[:, 8 * F:16 * F])
                        nc.vector.tensor_add(dA_t[:, :4 * F], dA_t[:, :4 * F], dA_t[:, 4 * F:8 * F])
                        nc.vector.tensor_add(dA_t[:, :2 * F], dA_t[:, :2 * F], dA_t[:, 2 * F:4 * F])
                        nc.vector.tensor_add(y_t[g][:, s0:s0 + F], dA_t[:, :F], dA_t[:, F:2 * F])
                # ---- stage O ----
                for c0 in range(0, Lb, MM):
                    F = min(MM, Lb - c0)
                    ztmp = cw.tile([128, MM], F32, tag="dtu", name="ztmp")
                    for g in range(2):
                        nc.vector.scalar_tensor_tensor(
                            y_t[g][:, c0:c0 + F], u_t[g][:, c0:c0 + F], vcol(g, NST),
                            y_t[g][:, c0:c0 + F], op0=Alu.mult, op1=Alu.add)
                        ps = mmp.tile([128, MM], F32, tag="mmps")
                        nc.tensor.matmul(ps[:, :F], winTb[:, (2 + g) * 128:(3 + g) * 128],
                                         x_ap[:, c0:c0 + F], start=True, stop=True)
                        nc.scalar.activation(ztmp[:, :F], ps[:, :F], Act.Sigmoid)
                        nc.vector.tensor_mul(y_t[g][:, c0:c0 + F], y_t[g][:, c0:c0 + F],
                                             ztmp[:, :F])
                        nc.scalar.activation(ztmp[:, :F], ps[:, :F], Act.Copy)
                        nc.vector.tensor_mul(y_t[g][:, c0:c0 + F], y_t[g][:, c0:c0 + F],
                                             ztmp[:, :F])
                    ps = mmp.tile([128, MM], F32, tag="mmps")
                    for g in range(2):
                        nc.tensor.matmul(ps[:, :F], woutTt[i][:, g * C:(g + 1) * C],
                                         y_t[g][:, c0:c0 + F], start=(g == 0), stop=(g == 1))
                    nc.scalar.activation(out_ap[:, c0:c0 + F], ps[:, :F], Act.Copy)
                    if out_dma is not None:
                        nc.sync.dma_start(out_dma[:, c0:c0 + F], out_ap[:, c0:c0 + F])

            def downconv(xt, off, j, Lb, out_ap):
                """xt: level tile; data at cols [off, off+Lb); front pad col off-1."""
                Lo = Lb // 2
                for c0 in range(0, Lo, MM):
                    F = min(MM, Lo - c0)
                    ps = mmp.tile([128, MM], F32, tag="mmps")
                    for k in range(3):
                        a = off + 2 * c0 + k - 1
                        nc.tensor.matmul(ps[:, :F], dcwTt[j][:, k * 128:(k + 1) * 128],
                                         xt[:, a:a + 2 * F - 1:2],
                                         start=(k == 0), stop=(k == 2))
                    nc.scalar.activation(out_ap[:, c0:c0 + F], ps[:, :F], Act.Identity,
                                         bias=gvecst[j][:, 0:1])

            def gate(t1_ap, t2_ap, j, Lb, f_ap):
                Fh = MM // 2
                for c0 in range(0, Lb, MM):   # output chunk
                    F = min(MM, Lb - c0)
                    ch = c0 // 2
                    Fi = F // 2
                    t2u = gw.tile([128, MM], F32, tag="t2u")
                    pse = mmp.tile([128, MM], F32, tag="mmps")
                    nc.tensor.matmul(pse[:, :Fi], upwt[j][:, :128],
                                     t2_ap[:, ch:ch + Fi], start=True, stop=True)
                    nc.scalar.activation(t2u[:, 0:F:2], pse[:, :Fi], Act.Identity,
                                         bias=gvecst[j][:, 1:2])
                    pso = mmp.tile([128, MM], F32, tag="mmps")
                    nc.tensor.matmul(pso[:, :Fi], upwt[j][:, 128:],
                                     t2_ap[:, ch:ch + Fi], start=True, stop=True)
                    nc.scalar.activation(t2u[:, 1:F:2], pso[:, :Fi], Act.Identity,
                                         bias=gvecst[j][:, 1:2])
                    ps = mmp.tile([128, MM], F32, tag="mmps")
                    nc.tensor.matmul(ps[:, :F], wgTt[j][:, :128], t1_ap[:, c0:c0 + F],
                                     start=True, stop=False)
                    nc.tensor.matmul(ps[:, :F], wgTt[j][:, 128:], t2u[:, :F],
                                     start=False, stop=True)
                    wloc = gw.tile([128, MM], F32, tag="wloc")
                    nc.scalar.activation(wloc[:, :F], ps[:, :F], Act.Sigmoid,
                                         bias=gvecst[j][:, 2:3])
                    m1 = gw.tile([128, MM], F32, tag="m1")
                    m2 = gw.tile([128, MM], F32, tag="m2")
                    nc.vector.tensor_mul(m1[:, :F], t1_ap[:, c0:c0 + F], wloc[:, :F])
                    nc.vector.tensor_mul(m2[:, :F], t2u[:, :F], wloc[:, :F])
                    nc.vector.tensor_sub(m2[:, :F], t2u[:, :F], m2[:, :F])
                    ps2 = mmp.tile([128, MM], F32, tag="mmps")
                    nc.tensor.matmul(ps2[:, :F], dbTt[j][:, :128], m1[:, :F],
                                     start=True, stop=False)
                    nc.tensor.matmul(ps2[:, :F], dbTt[j][:, 128:], m2[:, :F],
                                     start=False, stop=True)
                    nc.scalar.activation(f_ap[:, c0:c0 + F], ps2[:, :F], Act.Identity,
                                         bias=gvecst[j][:, 3:4])

            # ---------- network ----------
            x1 = lvl.tile([128, 1025], F32, tag="x1")
            x2 = lvl.tile([128, 513], F32, tag="x2")
            x3 = lvl.tile([128, 257], F32, tag="x3")
            x4 = lvl.tile([128, 128], F32, tag="x4")
            e1 = lvl.tile([128, 1024], F32, tag="e1")
            e2 = lvl.tile([128, 512], F32, tag="e2")
            e3 = lvl.tile([128, 256], F32, tag="e3")
            e4 = lvl.tile([128, 128], F32, tag="e4")
            d4 = lvl.tile([128, 256], F32, tag="x3", name="d4")
            d3 = lvl.tile([128, 512], F32, tag="x2", name="d3")
            fbuf = lvl.tile([128, 1024], F32, tag="fbuf")

            nc.vector.memset(xi[0][:, :3], 0.0)
            nc.vector.memset(xi[1][:, :3], 0.0)
            nc.vector.memset(x1[:, 0:1], 0.0)
            nc.vector.memset(x2[:, 0:1], 0.0)
            nc.vector.memset(x3[:, 0:1], 0.0)
            nc.sync.dma_start(x1[:, 1:1025], xT_d[:, :])

            mamba(x1[:, 1:1025], 0, 1024, e1[:, :])
            downconv(x1, 1, 0, 1024, x2[:, 1:513])
            mamba(x2[:, 1:513], 1, 512, e2[:, :])
            downconv(x2, 1, 1, 512, x3[:, 1:257])
            mamba(x3[:, 1:257], 2, 256, e3[:, :])
            downconv(x3, 1, 2, 256, x4[:, :])
            mamba(x4[:, :], 3, 128, e4[:, :])
            gate(e3[:, :], e4[:, :], 0, 256, fbuf[:, :256])
            mamba(fbuf[:, :256], 4, 256, d4[:, :])
            gate(e2[:, :], d4[:, :], 1, 512, fbuf[:, :512])
            mamba(fbuf[:, :512], 5, 512, d3[:, :])
            gate(e1[:, :], d3[:, :], 2, 1024, fbuf[:, :])
            d2 = x1  # x1 dead by now; reuse its slot
            mamba(fbuf[:, :], 6, 1024, d2[:, 1:1025], out_dma=out_d)

    nc.compile()
    return nc


def _get_program():
    if "nc" not in _CACHE:
        _CACHE["nc"] = _build()
    return _CACHE["nc"]


def kernel(**inputs):
    from concourse.bass_utils import run_bass_kernel_spmd

    nc = _get_program()
    w = _prep_weights(inputs)
    x = np.asarray(inputs["x"], np.float32)  # [B, L, C]
    in_maps = []
    for c in range(NCORES):
        m = {"xT": np.ascontiguousarray(x[c % B].T)}
        m.update(w)
        in_maps.append(m)
    res = run_bass_kernel_spmd(nc, in_maps, list(range(NCORES)))
    out = np.empty((B, L0, C), np.float32)
    for b in range(B):
        out[b] = res.results[b]["out"].T
    return out



# revision 9
# speedup vs baseline: 126.0387x; 126.0387x over previous
"""Trainium2 Bass kernel for the Mamba U-Net model (nn_Model_20770461843918).

Batch-data-parallel SPMD over 8 NeuronCores (4 batch elements; cores c and
c+4 duplicate work, outputs read from cores 0-3).  Per core the whole
7-block Mamba U-Net runs locally with partitions = inner channel d.

v2 layout (fp16 compute, fp32 PSUM / scan state):
  PE  : all matmuls in fp16 (1 cyc/col): in/x/dt/out projections, depthwise
        conv via diagonal matmuls, down/up/gate convs, K=1 ones-matmul
        broadcast of per-timestep B/C rows (shared across both d-groups)
  ACT : silu/exp/ln activations, PSUM->SBUF copies (table-swap minimized:
        stage M split into M1 in-proj / M2 conv+xproj / M3 dt passes)
  Pool: PSUM->SBUF fp16 copies of the B/C broadcasts (frees DVE 2x mode)
  DVE : dtu, dBu = dtu*Brep (1 instr, 3D AP), 16 tensor_tensor_scan
        (fp16 operands, fp32 internal state), h*Crep (1 instr), tree-reduce
"""
import numpy as np

B, L0, C = 4, 1024, 128
DI, NST, R, KC = 256, 16, 8, 4
NV = NST + 3          # packed per-partition vec cols: A[16], D, convb, bdt
NCORES = 8
TS = 512              # scan-stage time chunk
MM = 512              # matmul-stage time chunk

_CACHE = {}


def _prep_weights(inp):
    import ml_dtypes
    f32, f16 = np.float32, ml_dtypes.bfloat16
    g = lambda k: np.asarray(inp[k], f32)
    m_Win, m_convw, m_convb = g("m_Win"), g("m_convw"), g("m_convb")
    m_Wx, m_Wdt, m_bdt = g("m_Wx"), g("m_Wdt"), g("m_bdt")
    m_Alog, m_D, m_Wout = g("m_Alog"), g("m_D"), g("m_Wout")
    dc_w, dc_b = g("dc_w"), g("dc_b")
    wg_W, wg_b, db_W, db_b = g("wg_W"), g("wg_b"), g("db_W"), g("db_b")
    up_w, up_b = g("up_w"), g("up_b")

    winT = np.ascontiguousarray(m_Win.transpose(0, 2, 1)).astype(f16)  # [7,C,512]
    cd = np.zeros((7, 2, KC, 128, 128), f32)
    idx = np.arange(128)
    for i in range(7):
        for gg in range(2):
            for k in range(KC):
                cd[i, gg, k, idx, idx] = m_convw[i, gg * 128:(gg + 1) * 128, k]
    convdiag = np.ascontiguousarray(
        cd.transpose(0, 1, 3, 2, 4)).reshape(7, 2, 128, KC * 128).astype(f16)
    wxT_raw = np.ascontiguousarray(m_Wx.transpose(0, 2, 1)).reshape(7, 2, 128, R + 2 * NST)
    wxT = np.zeros((7, 2, 128, 64), f32)
    wxT[..., :R] = wxT_raw[..., :R]          # dt rows -> psum partitions 0..7
    wxT[..., 32:64] = wxT_raw[..., R:]       # B/C rows -> psum partitions 32..63
    wdtT = np.ascontiguousarray(m_Wdt.transpose(0, 2, 1))                # [7, R, DI]
    wdtall = wdtT.transpose(1, 0, 2).reshape(R, 7 * DI).astype(f16)     # [8, 7*256]
    woutT = np.ascontiguousarray(m_Wout.transpose(0, 2, 1)).reshape(7, 2, 128, C)
    dcwT = np.ascontiguousarray(dc_w.transpose(0, 2, 3, 1)).reshape(3, 128, 3 * 128)
    upw = np.ascontiguousarray(up_w.transpose(0, 1, 3, 2)).reshape(3, 128, 2 * 128)
    wgT = np.ascontiguousarray(wg_W.transpose(0, 2, 1)).reshape(3, 2, 128, 128)
    dbT = np.ascontiguousarray(db_W.transpose(0, 2, 1)).reshape(3, 2, 128, 128)

    # fp16 matmul-weight panels, order must match _build
    panels = []
    for i in range(7):
        panels += [wxT[i, 0], wxT[i, 1], woutT[i, 0], woutT[i, 1]]
    for j in range(3):
        panels += [dcwT[j], upw[j], wgT[j, 0], wgT[j, 1], dbT[j, 0], dbT[j, 1]]
    wtpack = np.concatenate(panels, axis=1).astype(f16)

    # fp32 per-partition scalar columns (act scale/bias, stt scalars)
    A = -np.exp(m_Alog)                                                  # [7, DI, N]
    vec = np.zeros((7, 2, 128, NV), f32)
    for gg in range(2):
        sl = slice(gg * 128, (gg + 1) * 128)
        vec[:, gg, :, :NST] = A[:, sl, :]
        vec[:, gg, :, NST] = m_D[:, sl]
        vec[:, gg, :, NST + 1] = m_convb[:, sl]
        vec[:, gg, :, NST + 2] = m_bdt[:, sl]
    gv = np.zeros((3, 128, 4), f32)
    gv[:, :, 0], gv[:, :, 1], gv[:, :, 2], gv[:, :, 3] = dc_b, up_b, wg_b, db_b
    vecpack = np.concatenate(
        [vec.transpose(0, 1, 3, 2).reshape(7 * 2 * NV, 128).T,
         gv.transpose(0, 2, 1).reshape(12, 128).T], axis=1)

    return {"winT": np.ascontiguousarray(winT),
            "convdiag": np.ascontiguousarray(convdiag),
            "wdtall": np.ascontiguousarray(wdtall),
            "wtpack": np.ascontiguousarray(wtpack),
            "vecpack": np.ascontiguousarray(vecpack.astype(f32))}


def _build():
    import concourse.bacc as bacc
    import concourse.tile as tile
    import concourse.mybir as mybir

    F32 = mybir.dt.float32
    F16 = mybir.dt.bfloat16
    Alu = mybir.AluOpType
    Act = mybir.ActivationFunctionType

    nc = bacc.Bacc("TRN2", target_bir_lowering=False, debug=False,
                   num_devices=NCORES)

    xT_d = nc.declare_dram_parameter("xT", [C, L0], F16, isOutput=False)
    out_d = nc.declare_dram_parameter("out", [C, L0], F32, isOutput=True)
    BLKW, GATW = 384, 1152
    TOTW = 7 * BLKW + 3 * GATW
    NVEC = 7 * 2 * NV + 3 * 4
    dram = {}
    for name, shape, dt in [
        ("winT", [7, C, 2 * DI], F16), ("convdiag", [7, 2, 128, KC * 128], F16),
        ("wdtall", [R, 7 * DI], F16), ("wtpack", [128, TOTW], F16),
        ("vecpack", [128, NVEC], F32),
    ]:
        dram[name] = nc.declare_dram_parameter(name, shape, dt, isOutput=False)
    bc_dram2 = [nc.dram_tensor("bc_bounce0", [2 * NST, L0], F16),
                nc.dram_tensor("bc_bounce1", [2 * NST, L0], F16)]

    with tile.TileContext(nc) as tc:
        with tc.tile_pool(name="wt", bufs=1) as wt, \
             tc.tile_pool(name="lvl", bufs=1) as lvl, \
             tc.tile_pool(name="blk", bufs=1) as blk, \
             tc.tile_pool(name="cube", bufs=1) as cube, \
             tc.tile_pool(name="cw", bufs=2) as cw, \
             tc.tile_pool(name="ubuf", bufs=2) as ubuf, \
             tc.tile_pool(name="gw", bufs=2) as gw, \
             tc.tile_pool(name="cwc", bufs=2) as cwc, \
             tc.tile_pool(name="bczp", bufs=1) as bczp, \
             tc.tile_pool(name="mbp", bufs=1) as mbp, \
             tc.tile_pool(name="mmp", bufs=3, space="PSUM") as mmp, \
             tc.tile_pool(name="xdbp", bufs=1, space="PSUM") as xdbp, \
             tc.tile_pool(name="repp", bufs=2, space="PSUM") as repp:

            ones2 = wt.tile([65, 128], F16, tag="ones2")
            nc.vector.memset(ones2[0:1, :], 1.0)
            nc.vector.memset(ones2[64:65, :], 1.0)

            def load_blk(i):
                winTb = cw.tile([C, 2 * DI], F16, tag="winT", name=f"winTb{i}")
                nc.scalar.dma_start(winTb[:], dram["winT"][i])
                cdw = cwc.tile([128, 2 * KC * 128], F16, tag="convdiag",
                               name=f"cdw{i}")
                nc.scalar.dma_start(cdw[:, :KC * 128], dram["convdiag"][i, 0])
                nc.scalar.dma_start(cdw[:, KC * 128:], dram["convdiag"][i, 1])
                return cdw, winTb

            preload = {0: load_blk(0)}

            wtall = wt.tile([128, TOTW], F16, tag="wtall")
            nc.scalar.dma_start(wtall[:, :TOTW // 2], dram["wtpack"][:, :TOTW // 2])
            nc.scalar.dma_start(wtall[:, TOTW // 2:], dram["wtpack"][:, TOTW // 2:])
            vecall = wt.tile([128, NVEC], F32, tag="vecall")
            nc.scalar.dma_start(vecall[:], dram["vecpack"][:])
            wdtall = wt.tile([R, 7 * DI], F16, tag="wdtall")
            nc.scalar.dma_start(wdtall[:], dram["wdtall"][:])
            wxTt, woutTt, wdtTt = [], [], []
            for i in range(7):
                o = i * BLKW
                wxTt.append(wtall[:, o:o + 128])
                woutTt.append(wtall[:, o + 128:o + BLKW])
                wdtTt.append(wdtall[:, i * DI:(i + 1) * DI])
            dcwTt, upwt, wgTt, dbTt = [], [], [], []
            for j in range(3):
                o = 7 * BLKW + j * GATW
                dcwTt.append(wtall[:, o:o + 384])
                upwt.append(wtall[:, o + 384:o + 640])
                wgTt.append(wtall[:, o + 640:o + 896])
                dbTt.append(wtall[:, o + 896:o + 1152])

            def vcol(i, g, c):
                o = i * 2 * NV + g * NV + c
                return vecall[:, o:o + 1]

            def gvcol(j, c):
                o = 7 * 2 * NV + j * 4 + c
                return vecall[:, o:o + 1]

            # per-block working tiles (double-buffered across blocks)
            def blk_tiles(i):
                xi = [blk.tile([128, L0 + 3], F16, tag=f"xi{g}",
                               name=f"xi{g}_{i}") for g in range(2)]
                z_t = [blk.tile([128, L0], F16, tag=f"z{g}", name=f"z{g}_{i}")
                       for g in range(2)]
                y_t = [blk.tile([128, L0], F16, tag=f"y{g}", name=f"y{g}_{i}")
                       for g in range(2)]
                xdbR = blk.tile([R, L0], F16, tag="xdbR", name=f"xdbR_{i}")
                bc16 = blk.tile([2 * NST, L0], F16, tag="bc16", name=f"bc16_{i}")
                u_t = [ubuf.tile([128, L0], F16, tag=f"u{g}", name=f"u{g}_{i}")
                       for g in range(2)]
                dt_t = [ubuf.tile([128, L0], F16, tag=f"dt{g}", name=f"dt{g}_{i}")
                        for g in range(2)]
                return xi, z_t, y_t, xdbR, bc16, u_t, dt_t

            carry = wt.tile([128, 2 * NST], F16, tag="carry")
            dA_g = [cube.tile([128, NST * TS], F16, tag=f"dA{g}",
                              name=f"dA{g}") for g in range(2)]
            dBu_g = [cube.tile([128, NST * TS], F16, tag=f"dBu{g}",
                               name=f"dBu{g}") for g in range(2)]
            repB = cube.tile([128, NST * TS], F16, tag="repB")
            repC = cube.tile([128, NST * TS], F16, tag="repC")

            def mamba(x_ap, i, Lb, out_ap, out_dma=None):
                cdw, winTb = preload.pop(i) if i in preload else load_blk(i)
                xi, z_t, y_t, xdbR, bc16, u_t, dt_t = blk_tiles(i)
                nc.vector.memset(xi[0][:, :3], 0.0)
                nc.vector.memset(xi[1][:, :3], 0.0)

                # ---- M1: in-proj (xi copies + z silus: one act table) ----
                for c0 in range(0, Lb, MM):
                    F = min(MM, Lb - c0)
                    for p in range(2):
                        ps = mmp.tile([128, MM], F32, tag="mmps")
                        nc.tensor.matmul(ps[:, :F], winTb[:, p * 128:(p + 1) * 128],
                                         x_ap[:, c0:c0 + F], start=True, stop=True)
                        nc.scalar.activation(xi[p][:, 3 + c0:3 + c0 + F],
                                             ps[:, :F], Act.Copy)
                    for g in range(2):
                        ps = mmp.tile([128, MM], F32, tag="mmps")
                        nc.tensor.matmul(ps[:, :F], winTb[:, (2 + g) * 128:(3 + g) * 128],
                                         x_ap[:, c0:c0 + F], start=True, stop=True)
                        nc.scalar.activation(z_t[g][:, c0:c0 + F], ps[:, :F],
                                             Act.Silu)
                # ---- M2: conv + u silu + x-proj (silu table) ----
                for c0 in range(0, Lb, MM):
                    F = min(MM, Lb - c0)
                    for g in range(2):
                        ps = mmp.tile([128, MM], F32, tag="mmps")
                        for k in range(KC):
                            nc.tensor.matmul(
                                ps[:, :F],
                                cdw[:, (g * KC + k) * 128:(g * KC + k + 1) * 128],
                                xi[g][:, c0 + k:c0 + k + F],
                                start=(k == 0), stop=(k == KC - 1))
                        nc.scalar.activation(u_t[g][:, c0:c0 + F], ps[:, :F],
                                             Act.Silu, bias=vcol(i, g, NST + 1))
                    psx = xdbp.tile([64, MM], F32, tag="xdbps")
                    for g in range(2):
                        nc.tensor.matmul(psx[:, :F],
                                         wxTt[i][:, g * 64:(g + 1) * 64],
                                         u_t[g][:, c0:c0 + F], start=(g == 0), stop=(g == 1))
                    nc.scalar.activation(xdbR[:, c0:c0 + F], psx[:R, :F], Act.Copy)
                    nc.scalar.activation(bc16[:, c0:c0 + F], psx[32:, :F], Act.Copy)
                    nc.sync.dma_start(bc_dram2[i % 2][:, c0:c0 + F], bc16[:, c0:c0 + F])
                # ---- M3: dt = softplus via exp+ln (nl_exp table) ----
                for c0 in range(0, Lb, MM):
                    F = min(MM, Lb - c0)
                    ztmp = cw.tile([128, MM], F16, tag="dtu", name="ztmpM")
                    for g in range(2):
                        ps = mmp.tile([128, MM], F32, tag="mmps")
                        nc.tensor.matmul(ps[:, :F], wdtTt[i][:, g * 128:(g + 1) * 128],
                                         xdbR[:, c0:c0 + F], start=True, stop=True)
                        nc.scalar.activation(ztmp[:, :F], ps[:, :F], Act.Exp,
                                             bias=vcol(i, g, NST + 2))
                        nc.scalar.activation(dt_t[g][:, c0:c0 + F], ztmp[:, :F],
                                             Act.Ln, bias=1.0)
                # ---- scan stage ----
                nchunks = (Lb + TS - 1) // TS
                for s in range(nchunks):
                    s0 = s * TS
                    F = min(TS, Lb - s0)
                    bc_dram = bc_dram2[i % 2]
                    bcz = bczp.tile([65, NST * TS], F16, tag="bcz")
                    nc.sync.dma_start(bcz[0:1, :NST * F], bc_dram[0:NST, s0:s0 + F])
                    nc.sync.dma_start(bcz[64:65, :NST * F], bc_dram[NST:, s0:s0 + F])
                    # broadcast B/C rows across partitions (shared by both g);
                    # ACT copies PSUM->SBUF fp16 so DVE muls run in 2x mode
                    for row, dst in ((0, repB), (64, repC)):
                        for n0 in range(0, NST, 2):
                            rp = repp.tile([128, 2 * TS], F32, tag="rep")
                            nc.tensor.matmul(rp[:, :F], ones2[row:row + 1, :],
                                             bcz[row:row + 1, n0 * F:(n0 + 1) * F],
                                             start=True, stop=True)
                            nc.tensor.matmul(rp[:, F:2 * F], ones2[row:row + 1, :],
                                             bcz[row:row + 1, (n0 + 1) * F:(n0 + 2) * F],
                                             start=True, stop=True)
                            nc.scalar.activation(dst[:, n0 * F:(n0 + 2) * F],
                                                 rp[:, :2 * F], Act.Copy)
                    for g in range(2):
                        dA_t, dBu_t = dA_g[g], dBu_g[g]
                        dtu = cw.tile([128, TS], F16, tag="dtu")
                        nc.vector.tensor_mul(dtu[:, :F], dt_t[g][:, s0:s0 + F],
                                             u_t[g][:, s0:s0 + F])
                        for n in range(NST):
                            nc.scalar.activation(dA_t[:, n * F:(n + 1) * F],
                                                 dt_t[g][:, s0:s0 + F], Act.Exp,
                                                 scale=vcol(i, g, n))
                        nc.vector.tensor_mul(
                            dBu_t[:, :NST * F].rearrange("p (a b) -> p a b", a=NST),
                            dtu[:, :F].unsqueeze(1).broadcast_to([128, NST, F]),
                            repB[:, :NST * F].rearrange("p (a b) -> p a b", a=NST))
                        for n in range(NST):
                            init = 0.0 if s == 0 else carry[:, g * NST + n:g * NST + n + 1]
                            nc.vector.tensor_tensor_scan(
                                dBu_t[:, n * F:(n + 1) * F],
                                dA_t[:, n * F:(n + 1) * F],
                                dBu_t[:, n * F:(n + 1) * F],
                                init, op0=Alu.mult, op1=Alu.add)
                        if s + 1 < nchunks:
                            nc.vector.tensor_copy(carry[:, g * NST:(g + 1) * NST],
                                                  dBu_t[:, F - 1:NST * F:F])
                        nc.vector.tensor_mul(
                            dA_t[:, :NST * F].rearrange("p (a b) -> p a b", a=NST),
                            dBu_t[:, :NST * F].rearrange("p (a b) -> p a b", a=NST),
                            repC[:, :NST * F].rearrange("p (a b) -> p a b", a=NST))
                        nc.vector.tensor_add(dA_t[:, :8 * F], dA_t[:, :8 * F], dA_t[:, 8 * F:16 * F])
                        nc.vector.tensor_add(dA_t[:, :4 * F], dA_t[:, :4 * F], dA_t[:, 4 * F:8 * F])
                        nc.vector.tensor_add(dA_t[:, :2 * F], dA_t[:, :2 * F], dA_t[:, 2 * F:4 * F])
                        nc.vector.tensor_add(y_t[g][:, s0:s0 + F], dA_t[:, :F], dA_t[:, F:2 * F])
                # ---- O: y = (y + u*D) * silu(z); out-proj ----
                for c0 in range(0, Lb, MM):
                    F = min(MM, Lb - c0)
                    ps = mmp.tile([128, MM], F32, tag="mmps")
                    for g in range(2):
                        nc.vector.scalar_tensor_tensor(
                            y_t[g][:, c0:c0 + F], u_t[g][:, c0:c0 + F],
                            vcol(i, g, NST),
                            y_t[g][:, c0:c0 + F], op0=Alu.mult, op1=Alu.add)
                        nc.vector.tensor_mul(y_t[g][:, c0:c0 + F], y_t[g][:, c0:c0 + F],
                                             z_t[g][:, c0:c0 + F])
                        nc.tensor.matmul(ps[:, :F], woutTt[i][:, g * C:(g + 1) * C],
                                         y_t[g][:, c0:c0 + F], start=(g == 0), stop=(g == 1))
                    nc.scalar.activation(out_ap[:, c0:c0 + F], ps[:, :F], Act.Copy)
                    if out_dma is not None:
                        nc.sync.dma_start(out_dma[:, c0:c0 + F], out_ap[:, c0:c0 + F])

            def downconv(xt, off, j, Lb, out_ap):
                """xt: level tile; data at cols [off, off+Lb); front pad col off-1."""
                Lo = Lb // 2
                for c0 in range(0, Lo, MM):
                    F = min(MM, Lo - c0)
                    ps = mmp.tile([128, MM], F32, tag="mmps")
                    for k in range(3):
                        a = off + 2 * c0 + k - 1
                        nc.tensor.matmul(ps[:, :F], dcwTt[j][:, k * 128:(k + 1) * 128],
                                         xt[:, a:a + 2 * F - 1:2],
                                         start=(k == 0), stop=(k == 2))
                    nc.scalar.activation(out_ap[:, c0:c0 + F], ps[:, :F], Act.Identity,
                                         bias=gvcol(j, 0))

            def gate(t1_ap, t2_ap, j, Lb, f_ap):
                for c0 in range(0, Lb, MM):   # output chunk
                    F = min(MM, Lb - c0)
                    ch = c0 // 2
                    Fi = F // 2
                    t2u = gw.tile([128, MM], F16, tag="t2u")
                    pse = mmp.tile([128, MM], F32, tag="mmps")
                    nc.tensor.matmul(pse[:, :Fi], upwt[j][:, :128],
                                     t2_ap[:, ch:ch + Fi], start=True, stop=True)
                    nc.scalar.activation(t2u[:, 0:F:2], pse[:, :Fi], Act.Identity,
                                         bias=gvcol(j, 1))
                    pso = mmp.tile([128, MM], F32, tag="mmps")
                    nc.tensor.matmul(pso[:, :Fi], upwt[j][:, 128:],
                                     t2_ap[:, ch:ch + Fi], start=True, stop=True)
                    nc.scalar.activation(t2u[:, 1:F:2], pso[:, :Fi], Act.Identity,
                                         bias=gvcol(j, 1))
                    ps = mmp.tile([128, MM], F32, tag="mmps")
                    nc.tensor.matmul(ps[:, :F], wgTt[j][:, :128], t1_ap[:, c0:c0 + F],
                                     start=True, stop=False)
                    nc.tensor.matmul(ps[:, :F], wgTt[j][:, 128:], t2u[:, :F],
                                     start=False, stop=True)
                    wloc = gw.tile([128, MM], F16, tag="wloc")
                    nc.scalar.activation(wloc[:, :F], ps[:, :F], Act.Sigmoid,
                                         bias=gvcol(j, 2))
                    m1 = gw.tile([128, MM], F16, tag="m1")
                    m2 = gw.tile([128, MM], F16, tag="m2")
                    nc.vector.tensor_mul(m1[:, :F], t1_ap[:, c0:c0 + F], wloc[:, :F])
                    nc.vector.tensor_mul(m2[:, :F], t2u[:, :F], wloc[:, :F])
                    nc.vector.tensor_sub(m2[:, :F], t2u[:, :F], m2[:, :F])
                    ps2 = mmp.tile([128, MM], F32, tag="mmps")
                    nc.tensor.matmul(ps2[:, :F], dbTt[j][:, :128], m1[:, :F],
                                     start=True, stop=False)
                    nc.tensor.matmul(ps2[:, :F], dbTt[j][:, 128:], m2[:, :F],
                                     start=False, stop=True)
                    nc.scalar.activation(f_ap[:, c0:c0 + F], ps2[:, :F], Act.Identity,
                                         bias=gvcol(j, 3))

            # ---------- network ----------
            x1 = lvl.tile([128, 1025], F16, tag="x1")
            x2 = lvl.tile([128, 513], F16, tag="x2")
            x3 = lvl.tile([128, 257], F16, tag="x3")
            x4 = lvl.tile([128, 128], F16, tag="x4")
            e1 = lvl.tile([128, 1024], F16, tag="e1")
            e2 = lvl.tile([128, 512], F16, tag="e2")
            e3 = lvl.tile([128, 256], F16, tag="e3")
            e4 = lvl.tile([128, 128], F16, tag="e4")
            d4 = lvl.tile([128, 256], F16, tag="x3", name="d4")
            d3 = lvl.tile([128, 512], F16, tag="x2", name="d3")
            fbuf = lvl.tile([128, 1024], F16, tag="fbuf")
            outb = lvl.tile([128, 1024], F32, tag="outb")

            nc.vector.memset(x1[:, 0:1], 0.0)
            nc.vector.memset(x2[:, 0:1], 0.0)
            nc.vector.memset(x3[:, 0:1], 0.0)
            nc.sync.dma_start(x1[:, 1:1025], xT_d[:, :])

            mamba(x1[:, 1:1025], 0, 1024, e1[:, :])
            downconv(x1, 1, 0, 1024, x2[:, 1:513])
            mamba(x2[:, 1:513], 1, 512, e2[:, :])
            downconv(x2, 1, 1, 512, x3[:, 1:257])
            mamba(x3[:, 1:257], 2, 256, e3[:, :])
            downconv(x3, 1, 2, 256, x4[:, :])
            mamba(x4[:, :], 3, 128, e4[:, :])
            gate(e3[:, :], e4[:, :], 0, 256, fbuf[:, :256])
            mamba(fbuf[:, :256], 4, 256, d4[:, :])
            gate(e2[:, :], d4[:, :], 1, 512, fbuf[:, :512])
            mamba(fbuf[:, :512], 5, 512, d3[:, :])
            gate(e1[:, :], d3[:, :], 2, 1024, fbuf[:, :])
            mamba(fbuf[:, :], 6, 1024, outb[:, :], out_dma=out_d)

            # ---- dead-code microbench: gpsimd op rates for next iteration ----
            mb1 = mbp.tile([128, 2048], F16, tag="mb1")
            mb2 = mbp.tile([128, 2048], F16, tag="mb2")
            mb3 = mbp.tile([128, 1024], F32, tag="mb3")
            nc.vector.memset(mb1[:, :], 0.5)
            nc.vector.memset(mb3[:, :], 0.25)
            nc.gpsimd.partition_broadcast(mb2[:, :2048], mb1[0:1, :2048])
            nc.gpsimd.tensor_mul(mb1[:, :2048], mb1[:, :2048], mb2[:, :2048])
            nc.vector.tensor_tensor_scan(mb3[:, :512], mb3[:, :512],
                                         mb3[:, 512:1024], 0.0,
                                         op0=Alu.mult, op1=Alu.add)

    nc.compile()
    return nc


def _get_program():
    if "nc" not in _CACHE:
        _CACHE["nc"] = _build()
    return _CACHE["nc"]


def kernel(**inputs):
    from concourse.bass_utils import run_bass_kernel_spmd

    nc = _get_program()
    w = _prep_weights(inputs)
    x = np.asarray(inputs["x"], np.float32)  # [B, L, C]
    in_maps = []
    for c in range(NCORES):
        import ml_dtypes
        m = {"xT": np.ascontiguousarray(x[c % B].T).astype(ml_dtypes.bfloat16)}
        m.update(w)
        in_maps.append(m)
    res = run_bass_kernel_spmd(nc, in_maps, list(range(NCORES)))
    out = np.empty((B, L0, C), np.float32)
    for b in range(B):
        out[b] = res.results[b]["out"].T
    return out
